# revision 31
# baseline (speedup 1.0000x reference)
"""Causal self-attention (B=2, S=2048, D=2048, H=16, Hd=128) on 8 trn2 cores.

Sharding: DP=2 over batch x TP=4 over heads. Core c handles batch b = c//4 and
global heads [4t, 4t+4) with t = c%4. Inputs are sharded/transposed on the
host with numpy; the full output y is assembled on the host from per-core
y^T slices.

Per-core SPMD program -- one software-pipelined loop over 512-col seq chunks,
attention computed entirely in the TRANSPOSED orientation (S^T[k, q]) so no
PE transposes are needed anywhere:
  - QKV projection (bf16, fp32 PSUM): qT/kT per head in (hd, seq) layout;
    v DIRECTLY in (seq, hd-features) layout by making the x-block the
    stationary operand (lhsT) instead of the weights. Chains are emitted
    2-wide (adjacent m-blocks alternate PSUM banks) so consecutive PE MMs
    never hit the same bank back-to-back (~127 vs ~148 ns/MM measured).
  - Per k-block j: S^T_j = k_j^T q in PSUM (one standalone matmul), exp on
    ACT (scale=1/sqrt(hd), no max subtraction -- fp32 exp cannot overflow)
    into bf16 P^T tiles, diagonal-block causal mask applied by gpsimd
    affine_select zero-fill AFTER exp.
  - Softmax normalization DEFERRED past PV: PV accumulates unnormalized
    P^T into pv[hd, q] PSUM (512-wide matmuls). Row sums l[q]: for chunks
    1-3 the P^T tiles are tree-reduced on DVE/Pool (pair sums + an
    in-place accumulate chain into a bf16 s tile, diag blocks folded in
    after their causal mask) so the PE pays only ONE [1,512] ones-matmul
    per head-chunk (~37k moving columns saved vs per-pair matmuls);
    chunk 0 (diag-only) keeps the direct per-block path. pv is scaled by
    broadcast 1/l during the PSUM->SBUF move (DVE).
  - ONE AllGather per seq chunk ([128, 2048] -> [512, 2048] over the 4-core
    TP group) of the per-chunk outT tile.
  - Software pipeline over chunks: chunk n's head loop interleaves, after
    each head, one proj chain of chunk n-1 AND one QKV chain-pair of chunk
    n+1 (the remaining two pairs run at chunk n+1's top). The chains are
    PE filler that hides the ACT exp critical path of the attention
    pipeline (exp of a [128,512] block costs ~400 ns on ACT vs ~260 ns of
    PE work per block) and lets each head's AllGather launch ~immediately
    (the pe_fill pop sites sit at kc==4 of the interleaved chains).
  - Chunk 3 (no following QKV phase): each head's normalize+AllGather is
    issued IMMEDIATELY after its last PV, and the final projection is
    emitted kc-major per head-group in gather-arrival order (head h's
    gather only feeds kc 4h..4h+3), holding the four m-block accumulators
    in 4 PSUM banks, so the last gathers overlap earlier proj chains;
    each finished m-slice is stored separately so only the last 512KB
    store is exposed.
  - Six dummy matmuls at t=0 pre-warm the PE HAM clock gate (1.2 -> 2.4
    GHz after ~3.4us of busy) during the initial DMA wait.
  - y^T is stored as bf16 (cast to fp32 on the host) to halve the final
    store tail.

DMA strategy: the HWDGE descriptor generator costs ~625ns per dma_start
regardless of size, so transfers are batched into wide multi-dim APs:
weights arrive in 1-4 dma_starts each (startup-ordered so QKV never
starves), x is double-buffered per chunk with a 1-dma prefetch, gathers/
projection loads/stores are 1-4 dma_starts per chunk.

PSUM: psW(2: QKV+proj) + psS(3: S^T pipeline) + psPV(2) + psL(1) = 8 banks.
The chunk-3 tail borrows 2 psS tiles (attention is drained by then) to hold
the 4 projection accumulators.
"""

import math
from contextlib import ExitStack

import numpy as np
import ml_dtypes

BF16_NP = ml_dtypes.bfloat16

import concourse.bass as bass
import concourse.mybir as mybir
import concourse.tile as tile
from concourse import bacc
from concourse.bass_utils import run_bass_kernel_spmd

FP32 = mybir.dt.float32
BF16 = mybir.dt.bfloat16

N_CORES = 8
TP = 4  # tensor-parallel group size (heads)
HPC = 4  # heads per core
B, S, D = 2, 2048, 2048
HD = 128
C_SCALE = 1.0 / math.sqrt(HD)
RG = [[0, 1, 2, 3], [4, 5, 6, 7]]
DEPTH = 3  # S^T matmuls issued ahead of PV in the PE stream

_NC_CACHE = {}


def build_nc(reps: int = 1, fake_collective: bool = False):
    key = (reps, fake_collective)
    if key in _NC_CACHE:
        return _NC_CACHE[key]
    nc = bacc.Bacc("TRN2", target_bir_lowering=False, debug=False, num_devices=N_CORES)

    xT_d = nc.declare_dram_parameter("xT", [D, S], BF16, isOutput=False)
    wqkT_d = nc.declare_dram_parameter("wqkT", [D, 2 * HPC * HD], BF16, isOutput=False)
    wvT_d = nc.declare_dram_parameter("wvT", [D, HPC * HD], BF16, isOutput=False)
    bqk_d = nc.declare_dram_parameter("bqk", [128, 2 * HPC], FP32, isOutput=False)
    bv_d = nc.declare_dram_parameter("bv", [1, HPC * HD], FP32, isOutput=False)
    woT_d = nc.declare_dram_parameter("woT", [D, HPC * HD], BF16, isOutput=False)
    bo_d = nc.declare_dram_parameter("bo", [128, HPC], FP32, isOutput=False)
    y_t_d = nc.declare_dram_parameter("y_t", [HPC * HD, S], BF16, isOutput=True)

    with tile.TileContext(nc, num_cores=N_CORES) as tc, ExitStack() as octx:
        cpool = octx.enter_context(tc.tile_pool(name="const", bufs=1))
        ones = cpool.tile([128, 1], BF16, tag="ones", name="ones")
        nc.gpsimd.memset(ones[:], 1.0)
        # 0/1 lower-triangle causal mask for diagonal 128x128 sub-blocks
        # (applied as a DVE multiply: the Pool queue also carries collective
        # triggers, which can block on peer handshakes, and a mask op stuck
        # behind one would stall the PV matmuls that consume it)
        ones_row = cpool.tile([1, 128], BF16, tag="ones_r", name="ones_r")
        nc.gpsimd.memset(ones_row[:], 1.0)
        mask01 = cpool.tile([128, 128], BF16, tag="mask01", name="mask01")
        nc.gpsimd.memset(mask01[:], 1.0)
        nc.gpsimd.affine_select(
            out=mask01[:], in_=mask01[:],
            pattern=[[1, 128]], compare_op=mybir.AluOpType.is_ge,
            fill=0.0, base=0, channel_multiplier=-1)
        bqk_sb = cpool.tile([128, 2 * HPC], FP32, tag="bqk", name="bqk")
        bv_sb = cpool.tile([1, HPC * HD], FP32, tag="bv", name="bv")
        bv_bc = cpool.tile([128, HPC * HD], FP32, tag="bvb", name="bvb")
        bo_sb = cpool.tile([128, HPC], FP32, tag="bo", name="bo")

        def load_bqk():  # tiny, needed by the first bias-add (~10us in)
            nc.sync.dma_start(out=bqk_sb[:], in_=bqk_d[:])

        def load_consts():  # remaining consts, needed ~30us+ in
            nc.sync.dma_start(out=bv_sb[:], in_=bv_d[:])
            nc.gpsimd.partition_broadcast(bv_bc[:], bv_sb[:])
            nc.sync.dma_start(out=bo_sb[:], in_=bo_d[:])

        for rep in range(reps):
            sfx = f"r{rep}"
            cc_in = [[nc.dram_tensor(f"cc_in{h}_{s}_{sfx}", [HD, S // 4], BF16)
                      for s in range(4)] for h in range(HPC)]
            cc_out = [[nc.dram_tensor(f"cc_out{h}_{s}_{sfx}", [TP * HD, S // 4], BF16)
                       for s in range(4)] for h in range(HPC)]
            _body(nc, tc, xT_d, wqkT_d, wvT_d, woT_d, y_t_d,
                  bqk_sb, bv_bc, bo_sb, ones, ones_row, mask01, cc_in, cc_out,
                  fake_collective,
                  (load_bqk, load_consts) if rep == 0 else None)

    nc.compile()
    _NC_CACHE[key] = nc
    return nc


def _body(nc, tc, xT_d, wqkT_d, wvT_d, woT_d, y_t_d,
          bqk_sb, bv_bc, bo_sb, ones, ones_row, mask01, cc_in, cc_out,
          fake_collective=False, load_consts=None):
    with ExitStack() as ctx:
        qkv_pool = ctx.enter_context(tc.tile_pool(name="qkv", bufs=1))
        # qT (m 0-3) / kT (m 4-7) per local head: (hd=128, S) bf16
        qkT_sb = [qkv_pool.tile([128, S], BF16, tag=f"qk{m}", name=f"qk{m}")
                  for m in range(8)]
        # v in (seq-within-block=128, 16 blocks * 512 head-features) bf16
        v_all = qkv_pool.tile([128, 16 * HPC * HD], BF16, tag="vall", name="vall")

        wA = ctx.enter_context(tc.tile_pool(name="wA", bufs=1))
        wqk_all = wA.tile([128, 16 * 1024], BF16, tag="wqk", name="wqk")
        wv_all = wA.tile([128, 16 * 512], BF16, tag="wv", name="wv")
        wo_all = wA.tile([128, 16 * 512], BF16, tag="wo", name="wo")

        xpool = ctx.enter_context(tc.tile_pool(name="xA", bufs=2))
        outc_pool = ctx.enter_context(tc.tile_pool(name="outc", bufs=2))
        ptpool = ctx.enter_context(tc.tile_pool(name="pt", bufs=6))
        prpool = ctx.enter_context(tc.tile_pool(name="pr", bufs=3))
        rpool = ctx.enter_context(tc.tile_pool(name="rinv", bufs=1))
        spool = ctx.enter_context(tc.tile_pool(name="ls", bufs=1))
        rbpool = ctx.enter_context(tc.tile_pool(name="rinvb", bufs=1))
        gpool = ctx.enter_context(tc.tile_pool(name="gD", bufs=2))
        ypool = ctx.enter_context(tc.tile_pool(name="yD", bufs=1))

        psW = ctx.enter_context(tc.tile_pool(name="psW", bufs=2, space="PSUM"))
        psS = ctx.enter_context(tc.tile_pool(name="psS", bufs=3, space="PSUM"))
        psPV = ctx.enter_context(tc.tile_pool(name="psPV", bufs=2, space="PSUM"))
        psL = ctx.enter_context(tc.tile_pool(name="psL", bufs=1, space="PSUM"))

        def r3(ap, pattern, **kw):
            return ap.rearrange(pattern, **kw)

        # x double-buffer: tile n covers chunk n, prefetched one chunk ahead
        xtiles = {}

        def alloc_x(n):
            xtiles[n] = xpool.tile([128, 16 * 512], BF16, tag="xc", name=f"xc{n}")

        def load_x(n, kc_lo, kc_hi):
            ncol = slice(n * 512, (n + 1) * 512)
            nc.sync.dma_start(
                out=r3(xtiles[n][:], "p (kc c) -> p kc c", c=512)[:, kc_lo:kc_hi],
                in_=r3(xT_d[:, ncol], "(kc p) c -> p kc c", p=128)[:, kc_lo:kc_hi])

        # deferred non-PE tails (normalize + gather bundle per head), popped
        # a few PE instructions into the NEXT head/chunk so the pair sums
        # and DVE never stall the PE
        pe_fill = []
        outc = {}
        gts = {}
        yts = {}

        def emit_head(h, n, immediate_tail=False):
            L = 4 * n + 4   # k-blocks
            pv = psPV.tile([128, 512], FP32, tag="pv", name="pv")
            lps = psL.tile([128, 512], FP32, tag="l", name="l")
            pts = {}
            prs = {}
            # l row-sums: for n>=1 the P^T tiles are tree-reduced on DVE/Pool
            # (pair sums + an in-place accumulate chain into s) so the PE
            # only pays ONE [1,512] ones-matmul per head-chunk instead of
            # one per pair + one per diagonal block (~37k moving columns
            # saved kernel-wide). Chunk 0 (diag-only, 4 blocks) keeps the
            # direct per-block path.
            # all tree ops on DVE: Pool also serves affine_select, collective
            # triggers and broadcasts, and a late tree op there would stall
            # the head-final l-matmul (and with it the PE queue)
            s = [None]
            eng_t = nc.vector

            def emit_st(j):
                qlo = max(0, j - 4 * n) * 128
                St = psS.tile([128, 512], FP32, tag="st", name="St")
                nc.tensor.matmul(
                    St[:, qlo:], qkT_sb[HPC + h][:, j * 128:(j + 1) * 128],
                    qkT_sb[h][:, n * 512 + qlo:(n + 1) * 512],
                    start=True, stop=True)
                PT = ptpool.tile([128, 512], BF16, tag="pt", name="PT")
                nc.scalar.activation(
                    PT[:, qlo:], St[:, qlo:],
                    mybir.ActivationFunctionType.Exp, bias=0.0, scale=C_SCALE)
                if j >= 4 * n:  # diagonal block: zero where k > q
                    nc.vector.tensor_mul(
                        PT[:, qlo:qlo + 128], PT[:, qlo:qlo + 128], mask01[:])
                    # fold masked diag block into the l tree (chunk 0 has no
                    # off-diag pairs, so its first diag tile initializes s)
                    if s[0] is None:
                        assert qlo == 0
                        s[0] = spool.tile([128, 512], BF16, tag="ls", name="ls")
                        eng_t.tensor_copy(s[0][:], PT[:])
                    else:
                        eng_t.tensor_add(s[0][:, qlo:], s[0][:, qlo:],
                                         PT[:, qlo:])
                pts[j] = (PT, qlo)
                if j % 2 == 1 and j < 4 * n:
                    # off-diag pair-sum PT_{j-1} + PT_j (independent adds)
                    i = j // 2
                    pr = prpool.tile([128, 512], BF16, tag="pr", name="pr")
                    nc.vector.tensor_add(pr[:], pts[j - 1][0][:], PT[:])
                    if i == 1:
                        s[0] = spool.tile([128, 512], BF16, tag="ls", name="ls")
                        eng_t.tensor_add(s[0][:], prs.pop(0)[:], pr[:])
                    elif i >= 2:
                        eng_t.tensor_add(s[0][:], s[0][:], pr[:])
                    else:
                        prs[i] = pr

            first_l = [True]

            def l_matmul(rhs_ap, is_last):
                nc.tensor.matmul(
                    lps[:1, 512 - rhs_ap.shape[1]:], ones[:], rhs_ap,
                    start=first_l[0], stop=is_last,
                    skip_group_check=not first_l[0])
                first_l[0] = False

            def emit_pv(j):
                PT, qlo = pts.pop(j)
                nc.tensor.matmul(
                    pv[:, qlo:], v_all[:, j * 512 + h * 128:j * 512 + (h + 1) * 128],
                    PT[:, qlo:], start=(j == 0), stop=(j == L - 1),
                    skip_group_check=(j != 0))

            for j in range(min(DEPTH, L)):
                emit_st(j)
            for j in range(L):
                emit_pv(j)
                if j == 1 and pe_fill:
                    pe_fill.pop(0)()
                if j + DEPTH < L:
                    emit_st(j + DEPTH)
            l_matmul(s[0][:], True)

            def do_tail():
                if n == 3:
                    # 1/l broadcast on the PE (ones-column x rinv-row into
                    # PSUM): the Pool partition_broadcast would queue behind
                    # the previous head's collective trigger, adding a
                    # peer-handshake wait to the tail's critical path
                    rinv = rpool.tile([1, 512], BF16, tag="rinv8", name="rinv8")
                    with nc.allow_low_precision(reason="1/l bcast row in bf16"):
                        nc.vector.reciprocal(rinv[:], lps[:1, :])
                    rbp = psL.tile([128, 512], FP32, tag="l", name="rbp")
                    nc.tensor.matmul(rbp[:], ones_row[:], rinv[:],
                                     start=True, stop=True)
                    rinvb = rbpool.tile([128, 512], FP32, tag="rinvb",
                                        name="rinvb")
                    nc.vector.tensor_copy(rinvb[:], rbp[:])
                    nc.vector.tensor_mul(
                        outc[n][:, h * 512:(h + 1) * 512], pv[:], rinvb[:])
                else:
                    rinv = rpool.tile([1, 512], FP32, tag="rinv", name="rinv")
                    nc.vector.reciprocal(rinv[:], lps[:1, :])
                    rinvb = rbpool.tile([128, 512], FP32, tag="rinvb",
                                        name="rinvb")
                    nc.gpsimd.partition_broadcast(rinvb[:], rinv[:])
                    nc.vector.tensor_mul(
                        outc[n][:, h * 512:(h + 1) * 512], pv[:], rinvb[:])
                nc.sync.dma_start(out=cc_in[h][n][:],
                                  in_=outc[n][:, h * 512:(h + 1) * 512])
                if fake_collective:
                    for rr in range(TP):
                        eng = nc.sync if rr % 2 == 0 else nc.scalar
                        eng.dma_start(
                            out=cc_out[h][n][rr * HD:(rr + 1) * HD, :],
                            in_=cc_in[h][n][:])
                else:
                    nc.gpsimd.collective_compute(
                        "AllGather", mybir.AluOpType.bypass,
                        replica_groups=RG,
                        ins=[cc_in[h][n][:]], outs=[cc_out[h][n][:]])
                # per-head projection loads: gathered row r*128+i of head-block
                # h  <->  gt column (h*4+r)*512+i. Lazy per-chunk gt tile so
                # the last head's loads pipeline into the projection. Loads
                # split across the SP and ACT HWDGE queues (2 independent
                # descriptor generators).
                if n not in gts:
                    gts[n] = gpool.tile([128, 16 * 512], BF16, tag="gt",
                                        name=f"gt{n}")
                for r in range(TP):
                    eng = nc.sync if r % 2 == 0 else nc.scalar
                    eng.dma_start(
                        out=gts[n][:, (h * 4 + r) * 512:(h * 4 + r + 1) * 512],
                        in_=cc_out[h][n][r * 128:(r + 1) * 128, :])

            if immediate_tail:
                do_tail()
            else:
                pe_fill.append(do_tail)

        def proj_chain(n, m):
            # one m-block (128 output features x 512 seq) of chunk n's
            # projection: a 16-MM accumulation chain + bias-activation; the
            # chain is PE filler between attention heads.
            if n not in yts:
                yts[n] = ypool.tile([128, 2048], BF16, tag="yt", name=f"yt{n}")
            gt = gts[n]
            psy = psW.tile([128, 512], FP32, tag="w512", name="py")
            for kc in range(16):
                nc.tensor.matmul(
                    psy[:], wo_all[:, kc * 512 + m * 128:kc * 512 + (m + 1) * 128],
                    gt[:, kc * 512:(kc + 1) * 512],
                    start=(kc == 0), stop=(kc == 15))
            nc.scalar.activation(
                yts[n][:, m * 512:(m + 1) * 512], psy[:],
                mybir.ActivationFunctionType.Identity,
                bias=bo_sb[:, m:m + 1], scale=1.0)
            if m == 3:
                store_y(n)

        def store_y(n):
            ncol_out = slice(n * 512, (n + 1) * 512)
            yt = yts.pop(n)
            gts.pop(n)
            nc.sync.dma_start(
                out=r3(y_t_d[:, ncol_out], "(m p) c -> p m c", p=128),
                in_=r3(yt[:], "p (m c) -> p m c", c=512))

        def proj_tail(n):
            # chunk-3 projection, kc-major per head-group in gather-arrival
            # order: head h's gather feeds exactly kc 4h..4h+3. The four
            # m-block accumulators live in 4 PSUM banks (2 psW + 2 borrowed
            # from the drained attention psS pool). Each m-slice is stored
            # as soon as its bias-activation completes (queues alternated)
            # so only the last 512KB store is exposed at the end.
            if n not in yts:
                yts[n] = ypool.tile([128, 2048], BF16, tag="yt", name=f"yt{n}")
            gt = gts[n]
            ncol_out = slice(n * 512, (n + 1) * 512)
            psy = [psW.tile([128, 512], FP32, tag="w512", name="pyA"),
                   psW.tile([128, 512], FP32, tag="w512", name="pyB"),
                   psS.tile([128, 512], FP32, tag="st", name="pyC"),
                   psS.tile([128, 512], FP32, tag="st", name="pyD")]
            for h in range(HPC):
                for kc in range(4 * h, 4 * h + 4):
                    for m in range(4):
                        nc.tensor.matmul(
                            psy[m][:],
                            wo_all[:, kc * 512 + m * 128:kc * 512 + (m + 1) * 128],
                            gt[:, kc * 512:(kc + 1) * 512],
                            start=(kc == 0), stop=(kc == 15))
            yt = yts.pop(n)
            gts.pop(n)
            for m in range(4):
                nc.scalar.activation(
                    yt[:, m * 512:(m + 1) * 512], psy[m][:],
                    mybir.ActivationFunctionType.Identity,
                    bias=bo_sb[:, m:m + 1], scale=1.0)
                eng = nc.sync if m % 2 == 0 else nc.scalar
                eng.dma_start(
                    out=y_t_d[m * 128:(m + 1) * 128, ncol_out],
                    in_=yt[:, m * 512:(m + 1) * 512])

        wqk3 = r3(wqk_all[:], "p (kc f) -> p kc f", f=1024)
        wqkd3 = r3(wqkT_d[:, :], "(kc p) f -> p kc f", p=128)

        def qkv_mm(ps, m, kc, xn):
            nc.tensor.matmul(
                ps[:], wqk_all[:, kc * 1024 + m * 128:kc * 1024 + (m + 1) * 128],
                xn[:, kc * 512:(kc + 1) * 512],
                start=(kc == 0), stop=(kc == 15))

        def v_mm(ps, sb, kc, xn):
            nc.tensor.matmul(
                ps[:], xn[:, kc * 512 + sb * 128:kc * 512 + (sb + 1) * 128],
                wv_all[:, kc * 512:(kc + 1) * 512],
                start=(kc == 0), stop=(kc == 15))

        def qkv_chains(n):
            # chunk n's QKV as six 2-wide chain thunks: four q/k m-block
            # pairs + two v seq-block pairs (2-wide so PSUM banks alternate
            # between consecutive PE instructions)
            xn = xtiles[n]
            chains = []

            def qk_pair(m0, xn=xn, n=n):
                psa = psW.tile([128, 512], FP32, tag="w512", name="psA")
                psb = psW.tile([128, 512], FP32, tag="w512", name="psB")
                for kc in range(16):
                    qkv_mm(psa, m0, kc, xn)
                    qkv_mm(psb, m0 + 1, kc, xn)
                    if kc == 4 and pe_fill:
                        pe_fill.pop(0)()
                nc.vector.tensor_scalar_add(
                    qkT_sb[m0][:, n * 512:(n + 1) * 512], psa[:],
                    bqk_sb[:, m0:m0 + 1])
                nc.vector.tensor_scalar_add(
                    qkT_sb[m0 + 1][:, n * 512:(n + 1) * 512], psb[:],
                    bqk_sb[:, m0 + 1:m0 + 2])

            def v_pair(sb0, xn=xn, n=n):
                psa = psW.tile([128, 512], FP32, tag="w512", name="psVA")
                psb = psW.tile([128, 512], FP32, tag="w512", name="psVB")
                for kc in range(16):
                    v_mm(psa, sb0, kc, xn)
                    v_mm(psb, sb0 + 1, kc, xn)
                    if kc == 4 and pe_fill:
                        pe_fill.pop(0)()
                for i, ps in ((0, psa), (1, psb)):
                    nc.vector.tensor_add(
                        v_all[:, (4 * n + sb0 + i) * 512:(4 * n + sb0 + i + 1) * 512],
                        ps[:], bv_bc[:])

            for m0 in range(0, 8, 2):
                chains.append(lambda m0=m0: qk_pair(m0))
            for sb0 in (0, 2):
                chains.append(lambda sb0=sb0: v_pair(sb0))
            return chains

        if load_consts is not None:
            # rep 0: pre-warm the PE HAM clock gate during the initial DMA
            # wait. The PE runs at 1.2 GHz until it has been busy ~3.4us;
            # six dummy N=512 matmuls (zeroed data, discarded) start that
            # clock at t~0 instead of at first-data-arrival (~2.7us), so
            # the real QKV stream starts at (or much closer to) 2.4 GHz.
            nc.gpsimd.memset(qkT_sb[0][:, 0:512], 0.0)
            for i in range(6):
                warm = psS.tile([128, 512], FP32, tag="st", name="warm")
                nc.tensor.matmul(warm[:1, :], ones[:],
                                 qkT_sb[0][:, 0:512], start=True, stop=True)

        # Software pipeline over chunks: chunk n's head loop interleaves one
        # QKV(n+1) chain after each head (PE filler hiding the ACT exp
        # path), with the remaining chains emitted at chunk n+1's top.
        chains_pending = []
        for n in range(4):  # seq chunks of 512
            if n == 0:
                # startup order: q-weights stream on the ACT HWDGE queue
                # while x0 kc-quarters stream in parallel on the SP queue,
                # then qk-bias (first bias-add ~10us in), k-weight halves
                # (m=4 ~18us in), other consts, v-weights, x1 prefetch,
                # o-weights
                alloc_x(0)
                for lo, hi in ((0, 2), (2, 4), (4, 8), (8, 12), (12, 16)):
                    nc.scalar.dma_start(out=wqk3[:, lo:hi, 0:512],
                                        in_=wqkd3[:, lo:hi, 0:512])
                    load_x(0, lo, hi)
                if load_consts is not None:
                    load_consts[0]()
                nc.scalar.dma_start(out=wqk3[:, 0:8, 512:1024],
                                    in_=wqkd3[:, 0:8, 512:1024])
                nc.scalar.dma_start(out=wqk3[:, 8:16, 512:1024],
                                    in_=wqkd3[:, 8:16, 512:1024])
                if load_consts is not None:
                    load_consts[1]()
                nc.scalar.dma_start(
                    out=r3(wv_all[:], "p (kc f) -> p kc f", f=512),
                    in_=r3(wvT_d[:, :], "(kc p) f -> p kc f", p=128))
                alloc_x(1)
                load_x(1, 0, 16)
                nc.scalar.dma_start(
                    out=r3(wo_all[:], "p (kc f) -> p kc f", f=512),
                    in_=r3(woT_d[:, :], "(kc p) f -> p kc f", p=128))
                chains_pending = qkv_chains(0)
            elif n < 3:  # prefetch next chunk's x (halves: less DMA blocking)
                alloc_x(n + 1)
                load_x(n + 1, 0, 8)
                load_x(n + 1, 8, 16)
            outc[n] = outc_pool.tile([128, HPC * 512], BF16, tag="outc",
                                     name=f"outc{n}")

            for c in chains_pending:
                c()
            nxt = qkv_chains(n + 1) if n < 3 else []

            for h in range(HPC):
                emit_head(h, n, immediate_tail=(n == 3))
                if n > 0:
                    # chunk n-1's projection: one chain per head as PE filler
                    proj_chain(n - 1, h)
                if nxt:
                    nxt.pop(0)()
            chains_pending = nxt

        proj_tail(3)


def make_in_maps(x, w_qkv, b_qkv, w_out, b_out):
    in_maps = []
    # gathered row g = h*512 + r*128 + i  <->  w_out column (4r+h)*128 + i
    dorder = np.array(
        [(4 * r + h) * 128 + i for h in range(HPC) for r in range(TP)
         for i in range(HD)])
    for c in range(N_CORES):
        b, t = divmod(c, TP)
        xT = np.ascontiguousarray(x[b].T)
        wq = w_qkv[512 * t:512 * (t + 1)]
        wk = w_qkv[D + 512 * t:D + 512 * (t + 1)]
        wv = w_qkv[2 * D + 512 * t:2 * D + 512 * (t + 1)]
        wqkT = np.ascontiguousarray(np.concatenate([wq, wk], axis=0).T)
        wvT = np.ascontiguousarray(wv.T)
        offs_qk = [512 * t + hh * 128 for hh in range(4)] + \
                  [D + 512 * t + hh * 128 for hh in range(4)]
        bqk = np.stack([b_qkv[o:o + 128] for o in offs_qk], axis=1)
        bv = np.ascontiguousarray(
            b_qkv[2 * D + 512 * t:2 * D + 512 * (t + 1)].reshape(1, 512))
        woT = np.ascontiguousarray(w_out[512 * t:512 * (t + 1)][:, dorder].T)
        bo = np.ascontiguousarray(b_out[512 * t:512 * (t + 1)].reshape(4, 128).T)
        in_maps.append(dict(
            xT=xT.astype(BF16_NP), wqkT=wqkT.astype(BF16_NP),
            wvT=wvT.astype(BF16_NP),
            bqk=np.ascontiguousarray(bqk), bv=bv,
            woT=woT.astype(BF16_NP), bo=bo))
    return in_maps


def assemble_y(results):
    y = np.empty((B, S, D), np.float32)
    for c in range(N_CORES):
        b, t = divmod(c, TP)
        y[b][:, 512 * t:512 * (t + 1)] = results[c]["y_t"].T.astype(np.float32)
    return y


def kernel(x, w_qkv, b_qkv, w_out, b_out):
    x = np.asarray(x, dtype=np.float32)
    w_qkv = np.asarray(w_qkv, dtype=np.float32)
    b_qkv = np.asarray(b_qkv, dtype=np.float32)
    w_out = np.asarray(w_out, dtype=np.float32)
    b_out = np.asarray(b_out, dtype=np.float32)

    nc = build_nc(1)
    in_maps = make_in_maps(x, w_qkv, b_qkv, w_out, b_out)
    r = run_bass_kernel_spmd(nc, in_maps, list(range(N_CORES)))
    return assemble_y(r.results)


# revision 33
# speedup vs baseline: 1.0175x; 1.0175x over previous
"""Causal self-attention (B=2, S=2048, D=2048, H=16, Hd=128) on 8 trn2 cores.

Sharding: DP=2 over batch x TP=4 over heads. Core c handles batch b = c//4 and
global heads [4t, 4t+4) with t = c%4. Inputs are sharded/transposed on the
host with numpy; the full output y is assembled on the host from per-core
y^T slices.

Per-core SPMD program -- one software-pipelined loop over 512-col seq chunks,
attention computed entirely in the TRANSPOSED orientation (S^T[k, q]) so no
PE transposes are needed anywhere:
  - QKV projection (bf16, fp32 PSUM): qT/kT per head in (hd, seq) layout;
    v DIRECTLY in (seq, hd-features) layout by making the x-block the
    stationary operand (lhsT) instead of the weights. Chains are emitted
    2-wide (adjacent m-blocks alternate PSUM banks) so consecutive PE MMs
    never hit the same bank back-to-back (~127 vs ~148 ns/MM measured).
  - Per k-block j: S^T_j = k_j^T q in PSUM (one standalone matmul), exp on
    ACT (scale=1/sqrt(hd), no max subtraction -- fp32 exp cannot overflow)
    into bf16 P^T tiles, diagonal-block causal mask applied AFTER exp as a
    DVE multiply by a constant 0/1 lower-triangle tile.
  - Softmax normalization DEFERRED past PV: PV accumulates unnormalized
    P^T into pv[hd, q] PSUM (512-wide matmuls). Row sums l[q]: the P^T
    tiles are tree-reduced on DVE (pair sums + an in-place accumulate
    chain into a bf16 s tile; diag blocks folded in after their causal
    mask, chunk 0's first diag tile initializes s by copy) so the PE pays
    only ONE [1,512] ones-matmul per head-chunk (~40k moving columns
    saved vs per-pair/per-block matmuls). pv is scaled by broadcast 1/l
    during the PSUM->SBUF move (DVE).
  - Engine placement rule: NOTHING that feeds the PE (masks, pair sums,
    l tree) runs on the Pool queue -- Pool also carries the AllGather
    triggers, which block on peer handshakes under core skew, and work
    queued behind them stalled the PE via the mask->PV and tree->l->PE
    chains (moving it to DVE measured ~45-80us/body faster under load).
    Pool keeps only collective triggers, 1/l partition-broadcasts and
    startup memsets.
  - ONE AllGather per seq chunk ([128, 2048] -> [512, 2048] over the 4-core
    TP group) of the per-chunk outT tile.
  - Software pipeline over chunks: chunk n's head loop interleaves, after
    each head, one proj chain of chunk n-1 AND one QKV chain-pair of chunk
    n+1 (the remaining two pairs run at chunk n+1's top). The chains are
    PE filler that hides the ACT exp critical path of the attention
    pipeline (exp of a [128,512] block costs ~400 ns on ACT vs ~260 ns of
    PE work per block) and lets each head's AllGather launch ~immediately
    (the pe_fill pop sites sit at kc==4 of the interleaved chains).
  - Chunk 3 (no following QKV phase): each head's normalize+AllGather is
    issued IMMEDIATELY after its last PV, and the final projection is
    emitted kc-major per head-group in gather-arrival order (head h's
    gather only feeds kc 4h..4h+3), holding the four m-block accumulators
    in 4 PSUM banks, so the last gathers overlap earlier proj chains;
    each finished m-slice is stored separately so only the last 512KB
    store is exposed.
  - Six dummy matmuls at t=0 pre-warm the PE HAM clock gate (1.2 -> 2.4
    GHz after ~3.4us of busy) during the initial DMA wait.
  - y^T is stored as bf16 (cast to fp32 on the host) to halve the final
    store tail.

DMA strategy: the HWDGE descriptor generator costs ~625ns per dma_start
regardless of size, so transfers are batched into wide multi-dim APs:
weights arrive in 1-4 dma_starts each (startup-ordered so QKV never
starves), x is double-buffered per chunk with a 1-dma prefetch, gathers/
projection loads/stores are 1-4 dma_starts per chunk.

PSUM: psW(2: QKV+proj) + psS(3: S^T pipeline) + psPV(2) + psL(1) = 8 banks.
The chunk-3 tail borrows 2 psS tiles (attention is drained by then) to hold
the 4 projection accumulators.
"""

import math
from contextlib import ExitStack

import numpy as np
import ml_dtypes

BF16_NP = ml_dtypes.bfloat16

import concourse.bass as bass
import concourse.mybir as mybir
import concourse.tile as tile
from concourse import bacc
from concourse.bass_utils import run_bass_kernel_spmd

FP32 = mybir.dt.float32
BF16 = mybir.dt.bfloat16

N_CORES = 8
TP = 4  # tensor-parallel group size (heads)
HPC = 4  # heads per core
B, S, D = 2, 2048, 2048
HD = 128
C_SCALE = 1.0 / math.sqrt(HD)
RG = [[0, 1, 2, 3], [4, 5, 6, 7]]
DEPTH = 3  # S^T matmuls issued ahead of PV in the PE stream

_NC_CACHE = {}


def build_nc(reps: int = 1, fake_collective: bool = False):
    key = (reps, fake_collective)
    if key in _NC_CACHE:
        return _NC_CACHE[key]
    nc = bacc.Bacc("TRN2", target_bir_lowering=False, debug=False, num_devices=N_CORES)

    xT_d = nc.declare_dram_parameter("xT", [D, S], BF16, isOutput=False)
    wqkT_d = nc.declare_dram_parameter("wqkT", [D, 2 * HPC * HD], BF16, isOutput=False)
    wvT_d = nc.declare_dram_parameter("wvT", [D, HPC * HD], BF16, isOutput=False)
    bqk_d = nc.declare_dram_parameter("bqk", [128, 2 * HPC], FP32, isOutput=False)
    bv_d = nc.declare_dram_parameter("bv", [1, HPC * HD], FP32, isOutput=False)
    woT_d = nc.declare_dram_parameter("woT", [D, HPC * HD], BF16, isOutput=False)
    bo_d = nc.declare_dram_parameter("bo", [128, HPC], FP32, isOutput=False)
    y_t_d = nc.declare_dram_parameter("y_t", [HPC * HD, S], BF16, isOutput=True)

    with tile.TileContext(nc, num_cores=N_CORES) as tc, ExitStack() as octx:
        cpool = octx.enter_context(tc.tile_pool(name="const", bufs=1))
        ones = cpool.tile([128, 1], BF16, tag="ones", name="ones")
        nc.gpsimd.memset(ones[:], 1.0)
        # 0/1 lower-triangle causal mask for diagonal 128x128 sub-blocks
        # (applied as a DVE multiply: the Pool queue also carries collective
        # triggers, which can block on peer handshakes, and a mask op stuck
        # behind one would stall the PV matmuls that consume it)
        mask01 = cpool.tile([128, 128], BF16, tag="mask01", name="mask01")
        nc.gpsimd.memset(mask01[:], 1.0)
        nc.gpsimd.affine_select(
            out=mask01[:], in_=mask01[:],
            pattern=[[1, 128]], compare_op=mybir.AluOpType.is_ge,
            fill=0.0, base=0, channel_multiplier=-1)
        bqk_sb = cpool.tile([128, 2 * HPC], FP32, tag="bqk", name="bqk")
        bv_sb = cpool.tile([1, HPC * HD], FP32, tag="bv", name="bv")
        bv_bc = cpool.tile([128, HPC * HD], FP32, tag="bvb", name="bvb")
        bo_sb = cpool.tile([128, HPC], FP32, tag="bo", name="bo")

        def load_bqk():  # tiny, needed by the first bias-add (~10us in)
            nc.sync.dma_start(out=bqk_sb[:], in_=bqk_d[:])

        def load_consts():  # remaining consts, needed ~30us+ in
            nc.sync.dma_start(out=bv_sb[:], in_=bv_d[:])
            nc.gpsimd.partition_broadcast(bv_bc[:], bv_sb[:])
            nc.sync.dma_start(out=bo_sb[:], in_=bo_d[:])

        for rep in range(reps):
            sfx = f"r{rep}"
            cc_in = [[nc.dram_tensor(f"cc_in{h}_{s}_{sfx}", [HD, S // 4], BF16)
                      for s in range(4)] for h in range(HPC)]
            cc_out = [[nc.dram_tensor(f"cc_out{h}_{s}_{sfx}", [TP * HD, S // 4], BF16)
                       for s in range(4)] for h in range(HPC)]
            _body(nc, tc, xT_d, wqkT_d, wvT_d, woT_d, y_t_d,
                  bqk_sb, bv_bc, bo_sb, ones, mask01, cc_in, cc_out,
                  fake_collective,
                  (load_bqk, load_consts) if rep == 0 else None)

    nc.compile()
    _NC_CACHE[key] = nc
    return nc


def _body(nc, tc, xT_d, wqkT_d, wvT_d, woT_d, y_t_d,
          bqk_sb, bv_bc, bo_sb, ones, mask01, cc_in, cc_out,
          fake_collective=False, load_consts=None):
    with ExitStack() as ctx:
        qkv_pool = ctx.enter_context(tc.tile_pool(name="qkv", bufs=1))
        # qT (m 0-3) / kT (m 4-7) per local head: (hd=128, S) bf16
        qkT_sb = [qkv_pool.tile([128, S], BF16, tag=f"qk{m}", name=f"qk{m}")
                  for m in range(8)]
        # v in (seq-within-block=128, 16 blocks * 512 head-features) bf16
        v_all = qkv_pool.tile([128, 16 * HPC * HD], BF16, tag="vall", name="vall")

        wA = ctx.enter_context(tc.tile_pool(name="wA", bufs=1))
        wqk_all = wA.tile([128, 16 * 1024], BF16, tag="wqk", name="wqk")
        wv_all = wA.tile([128, 16 * 512], BF16, tag="wv", name="wv")
        wo_all = wA.tile([128, 16 * 512], BF16, tag="wo", name="wo")

        xpool = ctx.enter_context(tc.tile_pool(name="xA", bufs=2))
        outc_pool = ctx.enter_context(tc.tile_pool(name="outc", bufs=2))
        ptpool = ctx.enter_context(tc.tile_pool(name="pt", bufs=6))
        prpool = ctx.enter_context(tc.tile_pool(name="pr", bufs=4))
        rpool = ctx.enter_context(tc.tile_pool(name="rinv", bufs=1))
        spool = ctx.enter_context(tc.tile_pool(name="ls", bufs=1))
        rbpool = ctx.enter_context(tc.tile_pool(name="rinvb", bufs=1))
        gpool = ctx.enter_context(tc.tile_pool(name="gD", bufs=2))
        ypool = ctx.enter_context(tc.tile_pool(name="yD", bufs=1))

        psW = ctx.enter_context(tc.tile_pool(name="psW", bufs=2, space="PSUM"))
        psS = ctx.enter_context(tc.tile_pool(name="psS", bufs=3, space="PSUM"))
        psPV = ctx.enter_context(tc.tile_pool(name="psPV", bufs=2, space="PSUM"))
        psL = ctx.enter_context(tc.tile_pool(name="psL", bufs=1, space="PSUM"))

        def r3(ap, pattern, **kw):
            return ap.rearrange(pattern, **kw)

        # x double-buffer: tile n covers chunk n, prefetched one chunk ahead
        xtiles = {}

        def alloc_x(n):
            xtiles[n] = xpool.tile([128, 16 * 512], BF16, tag="xc", name=f"xc{n}")

        def load_x(n, kc_lo, kc_hi):
            ncol = slice(n * 512, (n + 1) * 512)
            nc.sync.dma_start(
                out=r3(xtiles[n][:], "p (kc c) -> p kc c", c=512)[:, kc_lo:kc_hi],
                in_=r3(xT_d[:, ncol], "(kc p) c -> p kc c", p=128)[:, kc_lo:kc_hi])

        # deferred non-PE tails (normalize + gather bundle per head), popped
        # a few PE instructions into the NEXT head/chunk so the pair sums
        # and DVE never stall the PE
        pe_fill = []
        outc = {}
        gts = {}
        yts = {}

        def emit_head(h, n, immediate_tail=False):
            L = 4 * n + 4   # k-blocks
            pv = psPV.tile([128, 512], FP32, tag="pv", name="pv")
            lps = psL.tile([128, 512], FP32, tag="l", name="l")
            pts = {}
            prs = {}
            # l row-sums: for n>=1 the P^T tiles are tree-reduced on DVE/Pool
            # (pair sums + an in-place accumulate chain into s) so the PE
            # only pays ONE [1,512] ones-matmul per head-chunk instead of
            # one per pair + one per diagonal block (~37k moving columns
            # saved kernel-wide). Chunk 0 (diag-only, 4 blocks) keeps the
            # direct per-block path.
            # all tree ops on DVE: Pool also serves affine_select, collective
            # triggers and broadcasts, and a late tree op there would stall
            # the head-final l-matmul (and with it the PE queue)
            s = [None]
            eng_t = nc.vector

            def emit_st(j):
                qlo = max(0, j - 4 * n) * 128
                St = psS.tile([128, 512], FP32, tag="st", name="St")
                nc.tensor.matmul(
                    St[:, qlo:], qkT_sb[HPC + h][:, j * 128:(j + 1) * 128],
                    qkT_sb[h][:, n * 512 + qlo:(n + 1) * 512],
                    start=True, stop=True)
                PT = ptpool.tile([128, 512], BF16, tag="pt", name="PT")
                nc.scalar.activation(
                    PT[:, qlo:], St[:, qlo:],
                    mybir.ActivationFunctionType.Exp, bias=0.0, scale=C_SCALE)
                if j >= 4 * n:  # diagonal block: zero where k > q
                    nc.vector.tensor_mul(
                        PT[:, qlo:qlo + 128], PT[:, qlo:qlo + 128], mask01[:])
                    # fold masked diag block into the l tree (chunk 0 has no
                    # off-diag pairs, so its first diag tile initializes s)
                    if s[0] is None:
                        assert qlo == 0
                        s[0] = spool.tile([128, 512], BF16, tag="ls", name="ls")
                        eng_t.tensor_copy(s[0][:], PT[:])
                    else:
                        eng_t.tensor_add(s[0][:, qlo:], s[0][:, qlo:],
                                         PT[:, qlo:])
                pts[j] = (PT, qlo)
                if j % 2 == 1 and j < 4 * n:
                    # off-diag pair-sum PT_{j-1} + PT_j (independent adds)
                    i = j // 2
                    pr = prpool.tile([128, 512], BF16, tag="pr", name="pr")
                    nc.vector.tensor_add(pr[:], pts[j - 1][0][:], PT[:])
                    if i == 1:
                        s[0] = spool.tile([128, 512], BF16, tag="ls", name="ls")
                        eng_t.tensor_add(s[0][:], prs.pop(0)[:], pr[:])
                    elif i >= 2:
                        eng_t.tensor_add(s[0][:], s[0][:], pr[:])
                    else:
                        prs[i] = pr

            first_l = [True]

            def l_matmul(rhs_ap, is_last):
                nc.tensor.matmul(
                    lps[:1, 512 - rhs_ap.shape[1]:], ones[:], rhs_ap,
                    start=first_l[0], stop=is_last,
                    skip_group_check=not first_l[0])
                first_l[0] = False

            def emit_pv(j):
                PT, qlo = pts.pop(j)
                nc.tensor.matmul(
                    pv[:, qlo:], v_all[:, j * 512 + h * 128:j * 512 + (h + 1) * 128],
                    PT[:, qlo:], start=(j == 0), stop=(j == L - 1),
                    skip_group_check=(j != 0))

            for j in range(min(DEPTH, L)):
                emit_st(j)
            for j in range(L):
                emit_pv(j)
                if j == 1 and pe_fill:
                    pe_fill.pop(0)()
                if j + DEPTH < L:
                    emit_st(j + DEPTH)
            l_matmul(s[0][:], True)

            def do_tail():
                rinv = rpool.tile([1, 512], FP32, tag="rinv", name="rinv")
                nc.vector.reciprocal(rinv[:], lps[:1, :])
                rinvb = rbpool.tile([128, 512], FP32, tag="rinvb", name="rinvb")
                nc.gpsimd.partition_broadcast(rinvb[:], rinv[:])
                nc.vector.tensor_mul(
                    outc[n][:, h * 512:(h + 1) * 512], pv[:], rinvb[:])
                nc.sync.dma_start(out=cc_in[h][n][:],
                                  in_=outc[n][:, h * 512:(h + 1) * 512])
                if fake_collective:
                    for rr in range(TP):
                        eng = nc.sync if rr % 2 == 0 else nc.scalar
                        eng.dma_start(
                            out=cc_out[h][n][rr * HD:(rr + 1) * HD, :],
                            in_=cc_in[h][n][:])
                else:
                    nc.gpsimd.collective_compute(
                        "AllGather", mybir.AluOpType.bypass,
                        replica_groups=RG,
                        ins=[cc_in[h][n][:]], outs=[cc_out[h][n][:]])
                # per-head projection loads: gathered row r*128+i of head-block
                # h  <->  gt column (h*4+r)*512+i. Lazy per-chunk gt tile so
                # the last head's loads pipeline into the projection. Loads
                # split across the SP and ACT HWDGE queues (2 independent
                # descriptor generators).
                if n not in gts:
                    gts[n] = gpool.tile([128, 16 * 512], BF16, tag="gt",
                                        name=f"gt{n}")
                for r in range(TP):
                    eng = nc.sync if r % 2 == 0 else nc.scalar
                    eng.dma_start(
                        out=gts[n][:, (h * 4 + r) * 512:(h * 4 + r + 1) * 512],
                        in_=cc_out[h][n][r * 128:(r + 1) * 128, :])

            if immediate_tail:
                do_tail()
            else:
                pe_fill.append(do_tail)

        def proj_chain(n, m):
            # one m-block (128 output features x 512 seq) of chunk n's
            # projection: a 16-MM accumulation chain + bias-activation; the
            # chain is PE filler between attention heads.
            if n not in yts:
                yts[n] = ypool.tile([128, 2048], BF16, tag="yt", name=f"yt{n}")
            gt = gts[n]
            psy = psW.tile([128, 512], FP32, tag="w512", name="py")
            for kc in range(16):
                nc.tensor.matmul(
                    psy[:], wo_all[:, kc * 512 + m * 128:kc * 512 + (m + 1) * 128],
                    gt[:, kc * 512:(kc + 1) * 512],
                    start=(kc == 0), stop=(kc == 15))
            nc.scalar.activation(
                yts[n][:, m * 512:(m + 1) * 512], psy[:],
                mybir.ActivationFunctionType.Identity,
                bias=bo_sb[:, m:m + 1], scale=1.0)
            if m == 3:
                store_y(n)

        def store_y(n):
            ncol_out = slice(n * 512, (n + 1) * 512)
            yt = yts.pop(n)
            gts.pop(n)
            nc.sync.dma_start(
                out=r3(y_t_d[:, ncol_out], "(m p) c -> p m c", p=128),
                in_=r3(yt[:], "p (m c) -> p m c", c=512))

        def proj_tail(n):
            # chunk-3 projection, kc-major per head-group in gather-arrival
            # order: head h's gather feeds exactly kc 4h..4h+3. The four
            # m-block accumulators live in 4 PSUM banks (2 psW + 2 borrowed
            # from the drained attention psS pool). Each m-slice is stored
            # as soon as its bias-activation completes (queues alternated)
            # so only the last 512KB store is exposed at the end.
            if n not in yts:
                yts[n] = ypool.tile([128, 2048], BF16, tag="yt", name=f"yt{n}")
            gt = gts[n]
            ncol_out = slice(n * 512, (n + 1) * 512)
            psy = [psW.tile([128, 512], FP32, tag="w512", name="pyA"),
                   psW.tile([128, 512], FP32, tag="w512", name="pyB"),
                   psS.tile([128, 512], FP32, tag="st", name="pyC"),
                   psS.tile([128, 512], FP32, tag="st", name="pyD")]
            for h in range(HPC):
                for kc in range(4 * h, 4 * h + 4):
                    for m in range(4):
                        nc.tensor.matmul(
                            psy[m][:],
                            wo_all[:, kc * 512 + m * 128:kc * 512 + (m + 1) * 128],
                            gt[:, kc * 512:(kc + 1) * 512],
                            start=(kc == 0), stop=(kc == 15))
            yt = yts.pop(n)
            gts.pop(n)
            for m in range(4):
                nc.scalar.activation(
                    yt[:, m * 512:(m + 1) * 512], psy[m][:],
                    mybir.ActivationFunctionType.Identity,
                    bias=bo_sb[:, m:m + 1], scale=1.0)
                eng = nc.sync if m % 2 == 0 else nc.scalar
                eng.dma_start(
                    out=y_t_d[m * 128:(m + 1) * 128, ncol_out],
                    in_=yt[:, m * 512:(m + 1) * 512])

        wqk3 = r3(wqk_all[:], "p (kc f) -> p kc f", f=1024)
        wqkd3 = r3(wqkT_d[:, :], "(kc p) f -> p kc f", p=128)

        def qkv_mm(ps, m, kc, xn):
            nc.tensor.matmul(
                ps[:], wqk_all[:, kc * 1024 + m * 128:kc * 1024 + (m + 1) * 128],
                xn[:, kc * 512:(kc + 1) * 512],
                start=(kc == 0), stop=(kc == 15))

        def v_mm(ps, sb, kc, xn):
            nc.tensor.matmul(
                ps[:], xn[:, kc * 512 + sb * 128:kc * 512 + (sb + 1) * 128],
                wv_all[:, kc * 512:(kc + 1) * 512],
                start=(kc == 0), stop=(kc == 15))

        def qkv_chains(n):
            # chunk n's QKV as six 2-wide chain thunks: four q/k m-block
            # pairs + two v seq-block pairs (2-wide so PSUM banks alternate
            # between consecutive PE instructions)
            xn = xtiles[n]
            chains = []

            def qk_pair(m0, xn=xn, n=n):
                psa = psW.tile([128, 512], FP32, tag="w512", name="psA")
                psb = psW.tile([128, 512], FP32, tag="w512", name="psB")
                for kc in range(16):
                    qkv_mm(psa, m0, kc, xn)
                    qkv_mm(psb, m0 + 1, kc, xn)
                    if kc == 4 and pe_fill:
                        pe_fill.pop(0)()
                nc.vector.tensor_scalar_add(
                    qkT_sb[m0][:, n * 512:(n + 1) * 512], psa[:],
                    bqk_sb[:, m0:m0 + 1])
                nc.vector.tensor_scalar_add(
                    qkT_sb[m0 + 1][:, n * 512:(n + 1) * 512], psb[:],
                    bqk_sb[:, m0 + 1:m0 + 2])

            def v_pair(sb0, xn=xn, n=n):
                psa = psW.tile([128, 512], FP32, tag="w512", name="psVA")
                psb = psW.tile([128, 512], FP32, tag="w512", name="psVB")
                for kc in range(16):
                    v_mm(psa, sb0, kc, xn)
                    v_mm(psb, sb0 + 1, kc, xn)
                    if kc == 4 and pe_fill:
                        pe_fill.pop(0)()
                for i, ps in ((0, psa), (1, psb)):
                    nc.vector.tensor_add(
                        v_all[:, (4 * n + sb0 + i) * 512:(4 * n + sb0 + i + 1) * 512],
                        ps[:], bv_bc[:])

            for m0 in range(0, 8, 2):
                chains.append(lambda m0=m0: qk_pair(m0))
            for sb0 in (0, 2):
                chains.append(lambda sb0=sb0: v_pair(sb0))
            return chains

        if load_consts is not None:
            # rep 0: pre-warm the PE HAM clock gate during the initial DMA
            # wait. The PE runs at 1.2 GHz until it has been busy ~3.4us;
            # six dummy N=512 matmuls (zeroed data, discarded) start that
            # clock at t~0 instead of at first-data-arrival (~2.7us), so
            # the real QKV stream starts at (or much closer to) 2.4 GHz.
            nc.gpsimd.memset(qkT_sb[0][:, 0:512], 0.0)
            for i in range(6):
                warm = psS.tile([128, 512], FP32, tag="st", name="warm")
                nc.tensor.matmul(warm[:1, :], ones[:],
                                 qkT_sb[0][:, 0:512], start=True, stop=True)

        # Software pipeline over chunks: chunk n's head loop interleaves one
        # QKV(n+1) chain after each head (PE filler hiding the ACT exp
        # path), with the remaining chains emitted at chunk n+1's top.
        chains_pending = []
        for n in range(4):  # seq chunks of 512
            if n == 0:
                # startup order: q-weights stream on the ACT HWDGE queue
                # while x0 kc-quarters stream in parallel on the SP queue,
                # then qk-bias (first bias-add ~10us in), k-weight halves
                # (m=4 ~18us in), other consts, v-weights, x1 prefetch,
                # o-weights
                alloc_x(0)
                for lo, hi in ((0, 2), (2, 4), (4, 8), (8, 12), (12, 16)):
                    nc.scalar.dma_start(out=wqk3[:, lo:hi, 0:512],
                                        in_=wqkd3[:, lo:hi, 0:512])
                    load_x(0, lo, hi)
                if load_consts is not None:
                    load_consts[0]()
                nc.scalar.dma_start(out=wqk3[:, 0:8, 512:1024],
                                    in_=wqkd3[:, 0:8, 512:1024])
                nc.scalar.dma_start(out=wqk3[:, 8:16, 512:1024],
                                    in_=wqkd3[:, 8:16, 512:1024])
                if load_consts is not None:
                    load_consts[1]()
                nc.scalar.dma_start(
                    out=r3(wv_all[:], "p (kc f) -> p kc f", f=512),
                    in_=r3(wvT_d[:, :], "(kc p) f -> p kc f", p=128))
                alloc_x(1)
                load_x(1, 0, 16)
                nc.scalar.dma_start(
                    out=r3(wo_all[:], "p (kc f) -> p kc f", f=512),
                    in_=r3(woT_d[:, :], "(kc p) f -> p kc f", p=128))
                chains_pending = qkv_chains(0)
            elif n < 3:  # prefetch next chunk's x (halves: less DMA blocking)
                alloc_x(n + 1)
                load_x(n + 1, 0, 8)
                load_x(n + 1, 8, 16)
            outc[n] = outc_pool.tile([128, HPC * 512], BF16, tag="outc",
                                     name=f"outc{n}")

            for c in chains_pending:
                c()
            nxt = qkv_chains(n + 1) if n < 3 else []

            for h in range(HPC):
                emit_head(h, n, immediate_tail=(n == 3))
                if n > 0:
                    # chunk n-1's projection: one chain per head as PE filler
                    proj_chain(n - 1, h)
                if nxt:
                    nxt.pop(0)()
            chains_pending = nxt

        proj_tail(3)


def make_in_maps(x, w_qkv, b_qkv, w_out, b_out):
    in_maps = []
    # gathered row g = h*512 + r*128 + i  <->  w_out column (4r+h)*128 + i
    dorder = np.array(
        [(4 * r + h) * 128 + i for h in range(HPC) for r in range(TP)
         for i in range(HD)])
    for c in range(N_CORES):
        b, t = divmod(c, TP)
        xT = np.ascontiguousarray(x[b].T)
        wq = w_qkv[512 * t:512 * (t + 1)]
        wk = w_qkv[D + 512 * t:D + 512 * (t + 1)]
        wv = w_qkv[2 * D + 512 * t:2 * D + 512 * (t + 1)]
        wqkT = np.ascontiguousarray(np.concatenate([wq, wk], axis=0).T)
        wvT = np.ascontiguousarray(wv.T)
        offs_qk = [512 * t + hh * 128 for hh in range(4)] + \
                  [D + 512 * t + hh * 128 for hh in range(4)]
        bqk = np.stack([b_qkv[o:o + 128] for o in offs_qk], axis=1)
        bv = np.ascontiguousarray(
            b_qkv[2 * D + 512 * t:2 * D + 512 * (t + 1)].reshape(1, 512))
        woT = np.ascontiguousarray(w_out[512 * t:512 * (t + 1)][:, dorder].T)
        bo = np.ascontiguousarray(b_out[512 * t:512 * (t + 1)].reshape(4, 128).T)
        in_maps.append(dict(
            xT=xT.astype(BF16_NP), wqkT=wqkT.astype(BF16_NP),
            wvT=wvT.astype(BF16_NP),
            bqk=np.ascontiguousarray(bqk), bv=bv,
            woT=woT.astype(BF16_NP), bo=bo))
    return in_maps


def assemble_y(results):
    y = np.empty((B, S, D), np.float32)
    for c in range(N_CORES):
        b, t = divmod(c, TP)
        y[b][:, 512 * t:512 * (t + 1)] = results[c]["y_t"].T.astype(np.float32)
    return y


def kernel(x, w_qkv, b_qkv, w_out, b_out):
    x = np.asarray(x, dtype=np.float32)
    w_qkv = np.asarray(w_qkv, dtype=np.float32)
    b_qkv = np.asarray(b_qkv, dtype=np.float32)
    w_out = np.asarray(w_out, dtype=np.float32)
    b_out = np.asarray(b_out, dtype=np.float32)

    nc = build_nc(1)
    in_maps = make_in_maps(x, w_qkv, b_qkv, w_out, b_out)
    r = run_bass_kernel_spmd(nc, in_maps, list(range(N_CORES)))
    return assemble_y(r.results)
